# revision 36
# baseline (speedup 1.0000x reference)
"""Trainium2 Bass kernel for GQA causal prefill attention + KV-cache write.

Problem shapes (hardcoded): xq [2,1024,32,128] f32, xk/xv [2,1024,8,128] f32,
kv_buffer [2048,16,128] f32, cur_select_index [2048] int (arange).
Reference computes:
  kv_buffer_new = kv_buffer.at[idx].set(concat(xk, xv, axis=2).reshape(-1,16,128))
  out = causal_softmax(QK^T/sqrt(d)) @ V with GQA (4 q-heads per kv head),
        returned as [2,1024,4096].

Sharding: 8 cores, core m handles q-heads 4m..4m+3 (both batches) and
kv-head m. Each core runs 8 independent (batch, head) attention problems.

Per-core dataflow (matmuls in float32r: ~tf32 accuracy at bf16 PE rate,
measured end-to-end relmax error ~2.5e-4 vs the fp32 reference):
  - PE-transpose Q,K tiles -> Q^T,K^T [d,s] in SBUF (contraction needs d on
    partitions).
  - Scores computed transposed: per k-tile kt, S^T[k,q] = K_kt @ Q^T for
    q in [128*kt, 1024) only (causal block skipping), into a [128,<=1024]
    PSUM strip split into <=512 matmul pieces at the bank boundary.
  - One exp per strip on ScalarE with 1/sqrt(d) folded into the activation
    scale, PSUM -> packed SBUF tile E[128, kt, 1024]; causal masking of all
    eight 128x128 diagonal blocks with a single VectorE multiply (diagonal
    blocks all sit at strip offset 0).
  - O^T[d,q] += V_kt^T @ expS^T_kt with V in natural layout as stationary;
    denominator via ones-vector matmuls, accumulated per 512-wide q-group.
  - Denominator row PE-transposed to column layout, reciprocal on VectorE,
    normalization fused into the PSUM->SBUF copy after PE-transposing O^T
    back to [q,d].
"""

from contextlib import ExitStack

import numpy as np

import concourse.bacc as bacc
import concourse.bass as bass
import concourse.mybir as mybir
import concourse.tile as tile
from concourse.bass_utils import run_bass_kernel_spmd

F32 = mybir.dt.float32
F32R = mybir.dt.float32r
BF16 = mybir.dt.bfloat16
EXP = mybir.ActivationFunctionType.Exp

B = 2
S = 1024
HQ = 32
HKV = 8
D = 128
GROUP = HQ // HKV  # q-heads per kv head = heads per core per batch
NT = S // 128      # 8 s-tiles
N_CORES = 8
SCALE = 1.0 / float(np.sqrt(D))

# experiment flags
BF16_IDENT = False    # bf16 identity in PE transposes: rejected for f32 data
BCAST_NORM = True     # stride-0 broadcast tensor_tensor for normalization
OUT_DMA = "scalar"    # engine for output stores: "sync" | "scalar" | "gpsimd"
KV_DMA = "gpsimd"     # engine for kv dram->dram copies
LOAD_DMA = "sync"     # engine for input loads
STRIP1024 = True      # score strips [128,1024]x2 + 1 exp/kt; else [128,512]x3
MASK_GPSIMD = False   # run the causal mask multiply on gpsimd instead of DVE


def build_program():
    nc = bacc.Bacc("TRN2", target_bir_lowering=False, debug=False)

    xq = nc.dram_tensor("xq_s", [B, S, GROUP, D], F32, kind="ExternalInput")
    xk = nc.dram_tensor("xk_s", [B, S, D], F32, kind="ExternalInput")
    xv = nc.dram_tensor("xv_s", [B, S, D], F32, kind="ExternalInput")
    maskT = nc.dram_tensor("mask", [128, 128], F32, kind="ExternalInput")
    identT = nc.dram_tensor("ident", [128, 128], F32, kind="ExternalInput")
    out = nc.dram_tensor("out_s", [B, S, GROUP, D], F32, kind="ExternalOutput")
    kvk = nc.dram_tensor("kvk_s", [B * S, D], F32, kind="ExternalOutput")
    kvv = nc.dram_tensor("kvv_s", [B * S, D], F32, kind="ExternalOutput")

    IDT = BF16 if BF16_IDENT else F32

    with tile.TileContext(nc) as tc, ExitStack() as ctx:
        pool = lambda name, bufs, **kw: ctx.enter_context(
            tc.tile_pool(name=name, bufs=bufs, **kw))

        engs = {"sync": nc.sync, "scalar": nc.scalar, "gpsimd": nc.gpsimd}
        load_eng = engs[LOAD_DMA]
        kv_eng = engs[KV_DMA]
        out_eng = engs[OUT_DMA]

        consts = pool("consts", 1)
        knat_p = pool("knat", 1)
        vstg_p = pool("vstg", 1)
        v_p = pool("v", 2)
        kt_p = pool("kt", 2)
        qnat_p = pool("qnat", 2)
        qt_p = pool("qt", 2)
        e_p = pool("exps", 2)
        drow_p = pool("drow", 2)
        rec_p = pool("rec", 2)
        rb_p = pool("rb", 2)
        otsb_p = pool("otsb", 2)
        ofin_p = pool("ofin", 2)
        # PSUM budget is 8 banks:
        #   STRIP1024: sp 2x[128,1024]=4, tp 2x[128,512]=2, ot 1, dn 1
        #   else:      sp 3x[128,512]=3,  tp 2x[128,512]=2, ot 2, dn 1
        sp_ps = pool("sp", 2 if STRIP1024 else 3, space="PSUM")
        tp_ps = pool("tp", 2, space="PSUM")
        ot_ps = pool("ot", 1 if STRIP1024 else 2, space="PSUM")
        dn_ps = pool("dn", 1, space="PSUM")

        # --- PE warm-up: ~4us of dependency-free matmuls so the HAM clock
        # gate reaches K=8/8 before the real work arrives ---
        junk = consts.tile([128, 128], F32)
        nc.vector.memset(junk, 1.0)
        wu = sp_ps.tile([128, 1024], F32, tag="sp")
        for j in range(24):
            nc.tensor.matmul(wu[:, (j % 8) * 128:(j % 8) * 128 + 128],
                             junk[:], junk[:],
                             start=(j % 4 == 0), stop=(j % 4 == 3))

        # --- constants ---
        ident_sb = consts.tile([128, 128], F32)
        nc.sync.dma_start(out=ident_sb, in_=identT.ap())
        mask_sb = consts.tile([128, 128], F32)
        nc.sync.dma_start(out=mask_sb, in_=maskT.ap())
        ones_st = consts.tile([128, 32], F32)
        nc.vector.memset(ones_st, 1.0)
        ones_sb = consts.tile([128, 32], F32R)
        nc.vector.tensor_copy(ones_sb[:], ones_st[:])
        onesrow_st = consts.tile([1, 128], F32)
        nc.vector.memset(onesrow_st, 1.0)
        ones_row = consts.tile([1, 128], F32R)
        nc.vector.tensor_copy(ones_row[:], onesrow_st[:])

        def pe_transpose_1024(dst_sb, src_tiles):
            """PE-transpose eight [128,128] SBUF tiles through one 2-bank
            PSUM tile, then one DVE copy into dst_sb (with f32r rounding)."""
            tp = sp_ps.tile([128, 1024], F32, tag="sp")
            for g in range(2):
                for j in range(g * 4, g * 4 + 4):
                    nc.tensor.matmul(tp[:, j * 128:(j + 1) * 128],
                                     src_tiles[j], ident_sb[:],
                                     is_transpose=True, start=(j % 4 == 0),
                                     stop=(j % 4 == 3))
                # per-half cast: the first half drains to SBUF while the PE
                # transposes the second, so consumers start ~1us earlier
                nc.vector.tensor_copy(dst_sb[:, g * 512:(g + 1) * 512],
                                      tp[:, g * 512:(g + 1) * 512])

        def make_final_A(E, v_r):
            """PV + denominator accumulation, PSUM->SBUF copies."""
            state = {}

            def fin_a():
                otsb = otsb_p.tile([128, S], F32)   # O^T unnormalized
                dn = dn_ps.tile([32, 512], F32)
                drow = drow_p.tile([1, 1024], F32)
                for qg in range(2):
                    q0g = qg * 512
                    ot = ot_ps.tile([128, 512], F32, tag="ot")
                    kts = [kt for kt in range(NT) if kt * 128 < q0g + 512]
                    mm = []
                    for i, kt in enumerate(kts):
                        k0 = kt * 128
                        q0 = max(k0, q0g)
                        # E strip slice for q in [q0, q0g+512)
                        ex = E[:, kt, q0 - k0:q0g + 512 - k0]
                        mm.append((i == 0, i == len(kts) - 1, kt, q0 - q0g, ex))
                    for st, sp_, kt, co, ex in mm:
                        nc.tensor.matmul(ot[:, co:512], v_r[:, kt, :], ex,
                                         start=st, stop=sp_)
                    for st, sp_, kt, co, ex in mm:
                        nc.tensor.matmul(dn[:, co:512], ones_sb[:], ex,
                                         start=st, stop=sp_)
                    nc.vector.tensor_copy(otsb[:, q0g:q0g + 512], ot[:])
                    # stash this q-group's denominator row; the next group's
                    # start=True matmul re-zeroes the bank
                    nc.vector.tensor_copy(drow[0:1, q0g:q0g + 512],
                                          dn[0:1, :])
                state["otsb"] = otsb
                state["drow"] = drow
            return fin_a, state

        def make_final_B(state, b, h):
            """O^T transpose back to [q,d], reciprocal, normalize, store."""
            def fin_b():
                otsb, drow = state["otsb"], state["drow"]
                tps = []
                for g in range(2):
                    tp = tp_ps.tile([128, 512], F32, tag="tp")
                    for j in range(4):
                        t = g * 4 + j
                        nc.tensor.matmul(tp[:, j * 128:(j + 1) * 128],
                                         otsb[:, t * 128:(t + 1) * 128],
                                         ident_sb[:], is_transpose=True,
                                         start=(j == 0), stop=(j == 3))
                    tps.append(tp)

                dt = ot_ps.tile([128, 512], F32, tag="ot")
                for j in range(NT):
                    nc.tensor.matmul(dt[:, j:j + 1],
                                     drow[0:1, j * 128:(j + 1) * 128],
                                     ident_sb[0:1, 0:1],
                                     is_transpose=True, start=(j == 0),
                                     stop=(j == NT - 1))
                rec = rec_p.tile([128, NT], F32)
                nc.vector.reciprocal(rec[:], dt[:, 0:NT])

                ofin = ofin_p.tile([128, NT, D], F32)
                for g in range(2):
                    base = rec[:, g * 4:g * 4 + 4]
                    rec_b = bass.AP(tensor=base.tensor, offset=base.offset,
                                    ap=list(base.ap) + [[0, 128]])
                    nc.vector.tensor_mul(
                        ofin[:, g * 4:(g + 1) * 4, :],
                        tps[g][:].rearrange("p (t f) -> p t f", t=4), rec_b)
                out_eng.dma_start(
                    out=out.ap()[b, :, h, :].rearrange("(t p) d -> p t d", p=128),
                    in_=ofin)
            return fin_b

        def load_batch(b, eng):
            k_nat = knat_p.tile([128, NT, D], F32)
            eng.dma_start(
                out=k_nat, in_=xk.ap()[b].rearrange("(t p) d -> p t d", p=128))
            # kv-cache K write straight from SBUF: depends on the load, so
            # the scheduler can't hoist it into the startup HBM window
            kv_eng.dma_start(
                out=kvk.ap()[b * S:(b + 1) * S, :].rearrange(
                    "(t p) d -> p t d", p=128),
                in_=k_nat[:])
            return k_nat

        def transpose_k(k_nat):
            ktr = kt_p.tile([128, S], F32R)  # K^T: [d, k]
            pe_transpose_1024(ktr[:], [k_nat[:, j, :] for j in range(NT)])
            return ktr

        def load_v(b):
            # V load + f32r cast deferred: not needed until this batch's
            # first finalize, and an early DMA would flood the startup HBM
            # window / block the in-order DVE queue
            v_stg = vstg_p.tile([128, NT, D], F32)
            nc.scalar.dma_start(
                out=v_stg, in_=xv.ap()[b].rearrange("(t p) d -> p t d", p=128))
            v_r = v_p.tile([128, NT, D], F32R)
            nc.vector.tensor_copy(v_r[:], v_stg[:])
            kv_eng.dma_start(
                out=kvv.ap()[b * S:(b + 1) * S, :].rearrange(
                    "(t p) d -> p t d", p=128),
                in_=v_stg[:])
            return v_r

        def make_qtr(b, h):
            q_nat = qnat_p.tile([128, NT, D], F32)
            load_eng.dma_start(
                out=q_nat,
                in_=xq.ap()[b, :, h, :].rearrange("(t p) d -> p t d", p=128))
            qtr = qt_p.tile([128, S], F32R)  # Q^T: [d, q]
            pe_transpose_1024(qtr[:], [q_nat[:, j, :] for j in range(NT)])
            return qtr

        def emit_qk(ktr, qtr):
            # scores (transposed) + exp into packed E tile; strip kt covers
            # q in [k0, 1024), stored at E[:, kt, 0:W]
            E = e_p.tile([128, NT, 1024], F32R)
            for kt in range(NT):
                k0 = kt * 128
                W = 1024 - k0
                sp = sp_ps.tile([128, 1024], F32, tag="sp")
                for c0 in range(0, W, 512):  # pieces at psum bank boundary
                    w = min(512, W - c0)
                    nc.tensor.matmul(sp[:, c0:c0 + w], ktr[:, k0:k0 + 128],
                                     qtr[:, k0 + c0:k0 + c0 + w],
                                     start=True, stop=True)
                nc.scalar.activation(E[:, kt, 0:W], sp[:, 0:W], EXP,
                                     scale=SCALE)
            return E

        def emit_masks(E):
            # causal-mask the eight diagonal 128x128 blocks (two ops so
            # PV(qg0) doesn't wait on the kt>=4 exps)
            mask_eng = nc.gpsimd if MASK_GPSIMD else nc.vector
            mask_b4 = bass.AP(tensor=mask_sb.tensor, offset=mask_sb.offset,
                              ap=[mask_sb.ap[0], [0, 4], mask_sb.ap[1]])
            mask_eng.tensor_mul(E[:, 0:4, 0:128], E[:, 0:4, 0:128], mask_b4)
            mask_eng.tensor_mul(E[:, 4:8, 0:128], E[:, 4:8, 0:128], mask_b4)

        # Software pipeline, per head idx:
        #   PE order: QK(idx) | PV+dn(idx-1) | Qtrans(idx+1) | Otrans(idx-1)
        # so the DVE copies of finalize(idx-1) overlap PE transpose work and
        # the next head's Q^T cast overlaps the finalize matmuls.
        NH = B * GROUP
        ktrs = {0: transpose_k(load_batch(0, nc.sync))}
        k_next = None
        qtrs = {0: make_qtr(0, 0)}
        v_rs = {}
        a_pend, b_pend = [], []
        for idx in range(NH):
            b, h = divmod(idx, GROUP)
            if h == 0:
                v_rs[b] = load_v(b)
            if idx == 2 and B > 1:
                k_next = load_batch(1, nc.scalar)

            if idx == 0:
                # head 1's Q^T up front so head 0's QK doesn't wait on the
                # Q^T cast with an empty pipeline
                qtrs[1] = make_qtr(0, 1)
            E = emit_qk(ktrs[b], qtrs.pop(idx))
            if a_pend:
                a_pend.pop(0)()
            if idx + 1 < NH and idx + 1 not in qtrs:
                nb = (idx + 1) // GROUP
                if nb != b:
                    ktrs[nb] = transpose_k(k_next)
                qtrs[idx + 1] = make_qtr(nb, (idx + 1) % GROUP)
            if b_pend:
                b_pend.pop(0)()
            emit_masks(E)

            fin_a, state = make_final_A(E, v_rs[b])
            a_pend.append(fin_a)
            b_pend.append(make_final_B(state, b, h))

        while a_pend:
            a_pend.pop(0)()
            b_pend.pop(0)()

    nc.compile()
    return nc


_NC = None


def _get_nc():
    global _NC
    if _NC is None:
        _NC = build_program()
    return _NC


def make_in_maps(xq, xk, xv):
    xq = np.ascontiguousarray(np.asarray(xq, dtype=np.float32))
    xk = np.ascontiguousarray(np.asarray(xk, dtype=np.float32))
    xv = np.ascontiguousarray(np.asarray(xv, dtype=np.float32))
    mask = np.triu(np.ones((128, 128), dtype=np.float32))  # mask[k,q]=1 iff q>=k
    ident = np.eye(128, dtype=np.float32)
    in_maps = []
    for m in range(N_CORES):
        in_maps.append({
            "xq_s": np.ascontiguousarray(xq[:, :, GROUP * m:GROUP * (m + 1), :]),
            "xk_s": np.ascontiguousarray(xk[:, :, m, :]),
            "xv_s": np.ascontiguousarray(xv[:, :, m, :]),
            "mask": mask,
            "ident": ident,
        })
    return in_maps


def assemble(results, kv_buffer, cur_select_index):
    out = np.empty((B, S, HQ, D), dtype=np.float32)
    kv_new = np.array(kv_buffer, dtype=np.float32, copy=True)
    idx = np.asarray(cur_select_index)
    for m in range(N_CORES):
        r = results[m]
        out[:, :, GROUP * m:GROUP * (m + 1), :] = r["out_s"]
        kv_new[idx, m, :] = r["kvk_s"]
        kv_new[idx, HKV + m, :] = r["kvv_s"]
    return out.reshape(B, S, HQ * D), kv_new


def kernel(xq, xk, xv, kv_buffer, cur_select_index):
    nc = _get_nc()
    in_maps = make_in_maps(xq, xk, xv)
    res = run_bass_kernel_spmd(nc, in_maps, core_ids=list(range(N_CORES)))
    return assemble(res.results, kv_buffer, cur_select_index)


# revision 37
# speedup vs baseline: 1.0410x; 1.0410x over previous
"""Trainium2 Bass kernel for GQA causal prefill attention + KV-cache write.

Problem shapes (hardcoded): xq [2,1024,32,128] f32, xk/xv [2,1024,8,128] f32,
kv_buffer [2048,16,128] f32, cur_select_index [2048] int (arange).
Reference computes:
  kv_buffer_new = kv_buffer.at[idx].set(concat(xk, xv, axis=2).reshape(-1,16,128))
  out = causal_softmax(QK^T/sqrt(d)) @ V with GQA (4 q-heads per kv head),
        returned as [2,1024,4096].

Sharding: 8 cores, core m handles q-heads 4m..4m+3 (both batches) and
kv-head m. Each core runs 8 independent (batch, head) attention problems.

Per-core dataflow (matmuls in float32r: ~tf32 accuracy at bf16 PE rate,
measured end-to-end relmax error ~2.5e-4 vs the fp32 reference):
  - PE-transpose Q,K tiles -> Q^T,K^T [d,s] in SBUF (contraction needs d on
    partitions).
  - Scores computed transposed: per k-tile kt, S^T[k,q] = K_kt @ Q^T for
    q in [128*kt, 1024) only (causal block skipping), into a [128,<=1024]
    PSUM strip split into <=512 matmul pieces at the bank boundary.
  - One exp per strip on ScalarE with 1/sqrt(d) folded into the activation
    scale, PSUM -> packed SBUF tile E[128, kt, 1024]; causal masking of all
    eight 128x128 diagonal blocks with a single VectorE multiply (diagonal
    blocks all sit at strip offset 0).
  - O^T[d,q] += V_kt^T @ expS^T_kt with V in natural layout as stationary;
    denominator via ones-vector matmuls, accumulated per 512-wide q-group.
  - Denominator row PE-transposed to column layout, reciprocal on VectorE,
    normalization fused into the PSUM->SBUF copy after PE-transposing O^T
    back to [q,d].
"""

from contextlib import ExitStack

import numpy as np

import concourse.bacc as bacc
import concourse.bass as bass
import concourse.mybir as mybir
import concourse.tile as tile
from concourse.bass_utils import run_bass_kernel_spmd

F32 = mybir.dt.float32
F32R = mybir.dt.float32r
BF16 = mybir.dt.bfloat16
EXP = mybir.ActivationFunctionType.Exp

B = 2
S = 1024
HQ = 32
HKV = 8
D = 128
GROUP = HQ // HKV  # q-heads per kv head = heads per core per batch
NT = S // 128      # 8 s-tiles
N_CORES = 8
SCALE = 1.0 / float(np.sqrt(D))

# experiment flags
BF16_IDENT = False    # bf16 identity in PE transposes: rejected for f32 data
BCAST_NORM = True     # stride-0 broadcast tensor_tensor for normalization
OUT_DMA = "scalar"    # engine for output stores: "sync" | "scalar" | "gpsimd"
KV_DMA = "gpsimd"     # engine for kv dram->dram copies
LOAD_DMA = "sync"     # engine for input loads
STRIP1024 = True      # score strips [128,1024]x2 + 1 exp/kt; else [128,512]x3
MASK_GPSIMD = False   # run the causal mask multiply on gpsimd instead of DVE


def build_program():
    nc = bacc.Bacc("TRN2", target_bir_lowering=False, debug=False)

    xq = nc.dram_tensor("xq_s", [B, S, GROUP, D], F32, kind="ExternalInput")
    xk = nc.dram_tensor("xk_s", [B, S, D], F32, kind="ExternalInput")
    xv = nc.dram_tensor("xv_s", [B, S, D], F32, kind="ExternalInput")
    maskT = nc.dram_tensor("mask", [128, 128], F32, kind="ExternalInput")
    identT = nc.dram_tensor("ident", [128, 128], F32, kind="ExternalInput")
    out = nc.dram_tensor("out_s", [B, S, GROUP, D], F32, kind="ExternalOutput")
    kvk = nc.dram_tensor("kvk_s", [B * S, D], F32, kind="ExternalOutput")
    kvv = nc.dram_tensor("kvv_s", [B * S, D], F32, kind="ExternalOutput")

    IDT = BF16 if BF16_IDENT else F32

    with tile.TileContext(nc) as tc, ExitStack() as ctx:
        pool = lambda name, bufs, **kw: ctx.enter_context(
            tc.tile_pool(name=name, bufs=bufs, **kw))

        engs = {"sync": nc.sync, "scalar": nc.scalar, "gpsimd": nc.gpsimd}
        load_eng = engs[LOAD_DMA]
        kv_eng = engs[KV_DMA]
        out_eng = engs[OUT_DMA]

        consts = pool("consts", 1)
        knat_p = pool("knat", 1)
        vstg_p = pool("vstg", 1)
        v_p = pool("v", 2)
        kt_p = pool("kt", 2)
        qnat_p = pool("qnat", 2)
        qt_p = pool("qt", 2)
        e_p = pool("exps", 2)
        drow_p = pool("drow", 2)
        rec_p = pool("rec", 2)
        rb_p = pool("rb", 2)
        otsb_p = pool("otsb", 2)
        ofin_p = pool("ofin", 2)
        # PSUM budget is 8 banks:
        #   STRIP1024: sp 2x[128,1024]=4, tp 2x[128,512]=2, ot 1, dn 1
        #   else:      sp 3x[128,512]=3,  tp 2x[128,512]=2, ot 2, dn 1
        sp_ps = pool("sp", 2 if STRIP1024 else 3, space="PSUM")
        tp_ps = pool("tp", 2, space="PSUM")
        ot_ps = pool("ot", 1 if STRIP1024 else 2, space="PSUM")
        dn_ps = pool("dn", 1, space="PSUM")

        # --- PE warm-up: ~4us of dependency-free matmuls so the HAM clock
        # gate reaches K=8/8 before the real work arrives ---
        junk = consts.tile([128, 128], F32)
        nc.vector.memset(junk, 1.0)
        wu = sp_ps.tile([128, 1024], F32, tag="sp")
        for j in range(24):
            nc.tensor.matmul(wu[:, (j % 8) * 128:(j % 8) * 128 + 128],
                             junk[:], junk[:],
                             start=(j % 4 == 0), stop=(j % 4 == 3))

        # --- constants ---
        ident_sb = consts.tile([128, 128], F32)
        nc.sync.dma_start(out=ident_sb, in_=identT.ap())
        mask_sb = consts.tile([128, 128], F32)
        nc.sync.dma_start(out=mask_sb, in_=maskT.ap())
        ones_st = consts.tile([128, 32], F32)
        nc.vector.memset(ones_st, 1.0)
        ones_sb = consts.tile([128, 32], F32R)
        nc.vector.tensor_copy(ones_sb[:], ones_st[:])
        onesrow_st = consts.tile([1, 128], F32)
        nc.vector.memset(onesrow_st, 1.0)
        ones_row = consts.tile([1, 128], F32R)
        nc.vector.tensor_copy(ones_row[:], onesrow_st[:])

        def pe_transpose_1024(dst_sb, src_tiles):
            """PE-transpose eight [128,128] SBUF tiles through one 2-bank
            PSUM tile, then one DVE copy into dst_sb (with f32r rounding)."""
            tp = sp_ps.tile([128, 1024], F32, tag="sp")
            for j, src in enumerate(src_tiles):
                nc.tensor.matmul(tp[:, j * 128:(j + 1) * 128], src, ident_sb[:],
                                 is_transpose=True, start=(j % 4 == 0),
                                 stop=(j % 4 == 3))
            nc.vector.tensor_copy(dst_sb, tp[:])

        def make_final_A(E, v_r):
            """PV + denominator accumulation, PSUM->SBUF copies."""
            state = {}

            def fin_a():
                otsb = otsb_p.tile([128, S], F32)   # O^T unnormalized
                dn = dn_ps.tile([32, 512], F32)
                drow = drow_p.tile([1, 1024], F32)
                for qg in range(2):
                    q0g = qg * 512
                    ot = ot_ps.tile([128, 512], F32, tag="ot")
                    kts = [kt for kt in range(NT) if kt * 128 < q0g + 512]
                    mm = []
                    for i, kt in enumerate(kts):
                        k0 = kt * 128
                        q0 = max(k0, q0g)
                        # E strip slice for q in [q0, q0g+512)
                        ex = E[:, kt, q0 - k0:q0g + 512 - k0]
                        mm.append((i == 0, i == len(kts) - 1, kt, q0 - q0g, ex))
                    for st, sp_, kt, co, ex in mm:
                        nc.tensor.matmul(ot[:, co:512], v_r[:, kt, :], ex,
                                         start=st, stop=sp_)
                    for st, sp_, kt, co, ex in mm:
                        nc.tensor.matmul(dn[:, co:512], ones_sb[:], ex,
                                         start=st, stop=sp_)
                    nc.vector.tensor_copy(otsb[:, q0g:q0g + 512], ot[:])
                    # stash this q-group's denominator row; the next group's
                    # start=True matmul re-zeroes the bank
                    nc.vector.tensor_copy(drow[0:1, q0g:q0g + 512],
                                          dn[0:1, :])
                state["otsb"] = otsb
                state["drow"] = drow
            return fin_a, state

        def make_final_B(state, b, h):
            """O^T transpose back to [q,d], reciprocal, normalize, store."""
            def fin_b():
                otsb, drow = state["otsb"], state["drow"]
                tps = []
                for g in range(2):
                    tp = tp_ps.tile([128, 512], F32, tag="tp")
                    for j in range(4):
                        t = g * 4 + j
                        nc.tensor.matmul(tp[:, j * 128:(j + 1) * 128],
                                         otsb[:, t * 128:(t + 1) * 128],
                                         ident_sb[:], is_transpose=True,
                                         start=(j == 0), stop=(j == 3))
                    tps.append(tp)

                dt = ot_ps.tile([128, 512], F32, tag="ot")
                for j in range(NT):
                    nc.tensor.matmul(dt[:, j:j + 1],
                                     drow[0:1, j * 128:(j + 1) * 128],
                                     ident_sb[0:1, 0:1],
                                     is_transpose=True, start=(j == 0),
                                     stop=(j == NT - 1))
                rec = rec_p.tile([128, NT], F32)
                nc.vector.reciprocal(rec[:], dt[:, 0:NT])

                ofin = ofin_p.tile([128, NT, D], F32)
                for g in range(2):
                    base = rec[:, g * 4:g * 4 + 4]
                    rec_b = bass.AP(tensor=base.tensor, offset=base.offset,
                                    ap=list(base.ap) + [[0, 128]])
                    nc.vector.tensor_mul(
                        ofin[:, g * 4:(g + 1) * 4, :],
                        tps[g][:].rearrange("p (t f) -> p t f", t=4), rec_b)
                out_eng.dma_start(
                    out=out.ap()[b, :, h, :].rearrange("(t p) d -> p t d", p=128),
                    in_=ofin)
            return fin_b

        def load_batch(b, eng):
            k_nat = knat_p.tile([128, NT, D], F32)
            eng.dma_start(
                out=k_nat, in_=xk.ap()[b].rearrange("(t p) d -> p t d", p=128))
            # kv-cache K write straight from SBUF: depends on the load, so
            # the scheduler can't hoist it into the startup HBM window
            kv_eng.dma_start(
                out=kvk.ap()[b * S:(b + 1) * S, :].rearrange(
                    "(t p) d -> p t d", p=128),
                in_=k_nat[:])
            return k_nat

        def transpose_k(k_nat):
            ktr = kt_p.tile([128, S], F32R)  # K^T: [d, k]
            pe_transpose_1024(ktr[:], [k_nat[:, j, :] for j in range(NT)])
            return ktr

        def load_v(b):
            # V load + f32r cast deferred: not needed until this batch's
            # first finalize, and an early DMA would flood the startup HBM
            # window / block the in-order DVE queue
            v_stg = vstg_p.tile([128, NT, D], F32)
            nc.scalar.dma_start(
                out=v_stg, in_=xv.ap()[b].rearrange("(t p) d -> p t d", p=128))
            v_r = v_p.tile([128, NT, D], F32R)
            nc.vector.tensor_copy(v_r[:], v_stg[:])
            kv_eng.dma_start(
                out=kvv.ap()[b * S:(b + 1) * S, :].rearrange(
                    "(t p) d -> p t d", p=128),
                in_=v_stg[:])
            return v_r

        def make_qtr(b, h):
            q_nat = qnat_p.tile([128, NT, D], F32)
            load_eng.dma_start(
                out=q_nat,
                in_=xq.ap()[b, :, h, :].rearrange("(t p) d -> p t d", p=128))
            qtr = qt_p.tile([128, S], F32R)  # Q^T: [d, q]
            pe_transpose_1024(qtr[:], [q_nat[:, j, :] for j in range(NT)])
            return qtr

        def emit_qk(ktr, qtr):
            # scores (transposed) + exp into packed E tile; strip kt covers
            # q in [k0, 1024), stored at E[:, kt, 0:W]
            E = e_p.tile([128, NT, 1024], F32R)
            for kt in range(NT):
                k0 = kt * 128
                W = 1024 - k0
                sp = sp_ps.tile([128, 1024], F32, tag="sp")
                for c0 in range(0, W, 512):  # pieces at psum bank boundary
                    w = min(512, W - c0)
                    nc.tensor.matmul(sp[:, c0:c0 + w], ktr[:, k0:k0 + 128],
                                     qtr[:, k0 + c0:k0 + c0 + w],
                                     start=True, stop=True)
                nc.scalar.activation(E[:, kt, 0:W], sp[:, 0:W], EXP,
                                     scale=SCALE)
            return E

        def emit_masks(E):
            # causal-mask the eight diagonal 128x128 blocks (two ops so
            # PV(qg0) doesn't wait on the kt>=4 exps)
            mask_eng = nc.gpsimd if MASK_GPSIMD else nc.vector
            mask_b4 = bass.AP(tensor=mask_sb.tensor, offset=mask_sb.offset,
                              ap=[mask_sb.ap[0], [0, 4], mask_sb.ap[1]])
            mask_eng.tensor_mul(E[:, 0:4, 0:128], E[:, 0:4, 0:128], mask_b4)
            mask_eng.tensor_mul(E[:, 4:8, 0:128], E[:, 4:8, 0:128], mask_b4)

        # Software pipeline, per head idx:
        #   PE order: QK(idx) | PV+dn(idx-1) | Qtrans(idx+1) | Otrans(idx-1)
        # so the DVE copies of finalize(idx-1) overlap PE transpose work and
        # the next head's Q^T cast overlaps the finalize matmuls.
        NH = B * GROUP
        ktrs = {0: transpose_k(load_batch(0, nc.sync))}
        k_next = None
        qtrs = {0: make_qtr(0, 0)}
        v_rs = {}
        a_pend, b_pend = [], []
        for idx in range(NH):
            b, h = divmod(idx, GROUP)
            if h == 0:
                v_rs[b] = load_v(b)
            if idx == 2 and B > 1:
                k_next = load_batch(1, nc.scalar)

            if idx == 0:
                # head 1's Q^T up front so head 0's QK doesn't wait on the
                # Q^T cast with an empty pipeline
                qtrs[1] = make_qtr(0, 1)
            E = emit_qk(ktrs[b], qtrs.pop(idx))
            if a_pend:
                a_pend.pop(0)()
            if idx + 1 < NH and idx + 1 not in qtrs:
                nb = (idx + 1) // GROUP
                if nb != b:
                    ktrs[nb] = transpose_k(k_next)
                qtrs[idx + 1] = make_qtr(nb, (idx + 1) % GROUP)
            if b_pend:
                b_pend.pop(0)()
            emit_masks(E)

            fin_a, state = make_final_A(E, v_rs[b])
            a_pend.append(fin_a)
            b_pend.append(make_final_B(state, b, h))

        while a_pend:
            a_pend.pop(0)()
            b_pend.pop(0)()

    nc.compile()
    return nc


_NC = None


def _get_nc():
    global _NC
    if _NC is None:
        _NC = build_program()
    return _NC


def make_in_maps(xq, xk, xv):
    xq = np.ascontiguousarray(np.asarray(xq, dtype=np.float32))
    xk = np.ascontiguousarray(np.asarray(xk, dtype=np.float32))
    xv = np.ascontiguousarray(np.asarray(xv, dtype=np.float32))
    mask = np.triu(np.ones((128, 128), dtype=np.float32))  # mask[k,q]=1 iff q>=k
    ident = np.eye(128, dtype=np.float32)
    in_maps = []
    for m in range(N_CORES):
        in_maps.append({
            "xq_s": np.ascontiguousarray(xq[:, :, GROUP * m:GROUP * (m + 1), :]),
            "xk_s": np.ascontiguousarray(xk[:, :, m, :]),
            "xv_s": np.ascontiguousarray(xv[:, :, m, :]),
            "mask": mask,
            "ident": ident,
        })
    return in_maps


def assemble(results, kv_buffer, cur_select_index):
    out = np.empty((B, S, HQ, D), dtype=np.float32)
    kv_new = np.array(kv_buffer, dtype=np.float32, copy=True)
    idx = np.asarray(cur_select_index)
    for m in range(N_CORES):
        r = results[m]
        out[:, :, GROUP * m:GROUP * (m + 1), :] = r["out_s"]
        kv_new[idx, m, :] = r["kvk_s"]
        kv_new[idx, HKV + m, :] = r["kvv_s"]
    return out.reshape(B, S, HQ * D), kv_new


def kernel(xq, xk, xv, kv_buffer, cur_select_index):
    nc = _get_nc()
    in_maps = make_in_maps(xq, xk, xv)
    res = run_bass_kernel_spmd(nc, in_maps, core_ids=list(range(N_CORES)))
    return assemble(res.results, kv_buffer, cur_select_index)


# revision 38
# speedup vs baseline: 1.0473x; 1.0061x over previous
"""Trainium2 Bass kernel for GQA causal prefill attention + KV-cache write.

Problem shapes (hardcoded): xq [2,1024,32,128] f32, xk/xv [2,1024,8,128] f32,
kv_buffer [2048,16,128] f32, cur_select_index [2048] int (arange).
Reference computes:
  kv_buffer_new = kv_buffer.at[idx].set(concat(xk, xv, axis=2).reshape(-1,16,128))
  out = causal_softmax(QK^T/sqrt(d)) @ V with GQA (4 q-heads per kv head),
        returned as [2,1024,4096].

Sharding: 8 cores, core m handles q-heads 4m..4m+3 (both batches) and
kv-head m. Each core runs 8 independent (batch, head) attention problems.

Per-core dataflow (matmuls in float32r: ~tf32 accuracy at bf16 PE rate,
measured end-to-end relmax error ~2.5e-4 vs the fp32 reference):
  - PE-transpose Q,K tiles -> Q^T,K^T [d,s] in SBUF (contraction needs d on
    partitions).
  - Scores computed transposed: per k-tile kt, S^T[k,q] = K_kt @ Q^T for
    q in [128*kt, 1024) only (causal block skipping), into a [128,<=1024]
    PSUM strip split into <=512 matmul pieces at the bank boundary.
  - One exp per strip on ScalarE with 1/sqrt(d) folded into the activation
    scale, PSUM -> packed SBUF tile E[128, kt, 1024]; causal masking of all
    eight 128x128 diagonal blocks with a single VectorE multiply (diagonal
    blocks all sit at strip offset 0).
  - O^T[d,q] += V_kt^T @ expS^T_kt with V in natural layout as stationary;
    denominator via ones-vector matmuls, accumulated per 512-wide q-group.
  - Denominator row PE-transposed to column layout, reciprocal on VectorE,
    normalization fused into the PSUM->SBUF copy after PE-transposing O^T
    back to [q,d].
"""

from contextlib import ExitStack

import numpy as np

import concourse.bacc as bacc
import concourse.bass as bass
import concourse.mybir as mybir
import concourse.tile as tile
from concourse.bass_utils import run_bass_kernel_spmd

F32 = mybir.dt.float32
F32R = mybir.dt.float32r
BF16 = mybir.dt.bfloat16
EXP = mybir.ActivationFunctionType.Exp

B = 2
S = 1024
HQ = 32
HKV = 8
D = 128
GROUP = HQ // HKV  # q-heads per kv head = heads per core per batch
NT = S // 128      # 8 s-tiles
N_CORES = 8
SCALE = 1.0 / float(np.sqrt(D))

# experiment flags
BF16_IDENT = False    # bf16 identity in PE transposes: rejected for f32 data
BCAST_NORM = True     # stride-0 broadcast tensor_tensor for normalization
OUT_DMA = "sync"    # engine for output stores: "sync" | "scalar" | "gpsimd"
KV_DMA = "gpsimd"     # engine for kv dram->dram copies
LOAD_DMA = "sync"     # engine for input loads
STRIP1024 = True      # score strips [128,1024]x2 + 1 exp/kt; else [128,512]x3
MASK_GPSIMD = False   # run the causal mask multiply on gpsimd instead of DVE


def build_program():
    nc = bacc.Bacc("TRN2", target_bir_lowering=False, debug=False)

    xq = nc.dram_tensor("xq_s", [B, S, GROUP, D], F32, kind="ExternalInput")
    xk = nc.dram_tensor("xk_s", [B, S, D], F32, kind="ExternalInput")
    xv = nc.dram_tensor("xv_s", [B, S, D], F32, kind="ExternalInput")
    maskT = nc.dram_tensor("mask", [128, 128], F32, kind="ExternalInput")
    identT = nc.dram_tensor("ident", [128, 128], F32, kind="ExternalInput")
    out = nc.dram_tensor("out_s", [B, S, GROUP, D], F32, kind="ExternalOutput")
    kvk = nc.dram_tensor("kvk_s", [B * S, D], F32, kind="ExternalOutput")
    kvv = nc.dram_tensor("kvv_s", [B * S, D], F32, kind="ExternalOutput")

    IDT = BF16 if BF16_IDENT else F32

    with tile.TileContext(nc) as tc, ExitStack() as ctx:
        pool = lambda name, bufs, **kw: ctx.enter_context(
            tc.tile_pool(name=name, bufs=bufs, **kw))

        engs = {"sync": nc.sync, "scalar": nc.scalar, "gpsimd": nc.gpsimd}
        load_eng = engs[LOAD_DMA]
        kv_eng = engs[KV_DMA]
        out_eng = engs[OUT_DMA]

        consts = pool("consts", 1)
        knat_p = pool("knat", 1)
        vstg_p = pool("vstg", 1)
        v_p = pool("v", 2)
        kt_p = pool("kt", 2)
        qnat_p = pool("qnat", 2)
        qt_p = pool("qt", 2)
        e_p = pool("exps", 2)
        drow_p = pool("drow", 2)
        rec_p = pool("rec", 2)
        rb_p = pool("rb", 2)
        otsb_p = pool("otsb", 2)
        ofin_p = pool("ofin", 2)
        # PSUM budget is 8 banks:
        #   STRIP1024: sp 2x[128,1024]=4, tp 2x[128,512]=2, ot 1, dn 1
        #   else:      sp 3x[128,512]=3,  tp 2x[128,512]=2, ot 2, dn 1
        sp_ps = pool("sp", 2 if STRIP1024 else 3, space="PSUM")
        tp_ps = pool("tp", 2, space="PSUM")
        ot_ps = pool("ot", 1 if STRIP1024 else 2, space="PSUM")
        dn_ps = pool("dn", 1, space="PSUM")

        # --- PE warm-up: ~4us of dependency-free matmuls so the HAM clock
        # gate reaches K=8/8 before the real work arrives ---
        junk = consts.tile([128, 128], F32)
        nc.vector.memset(junk, 1.0)
        wu = sp_ps.tile([128, 1024], F32, tag="sp")
        for j in range(24):
            nc.tensor.matmul(wu[:, (j % 8) * 128:(j % 8) * 128 + 128],
                             junk[:], junk[:],
                             start=(j % 4 == 0), stop=(j % 4 == 3))

        # --- constants ---
        ident_sb = consts.tile([128, 128], F32)
        nc.sync.dma_start(out=ident_sb, in_=identT.ap())
        mask_sb = consts.tile([128, 128], F32)
        nc.sync.dma_start(out=mask_sb, in_=maskT.ap())
        ones_st = consts.tile([128, 32], F32)
        nc.vector.memset(ones_st, 1.0)
        ones_sb = consts.tile([128, 32], F32R)
        nc.vector.tensor_copy(ones_sb[:], ones_st[:])
        onesrow_st = consts.tile([1, 128], F32)
        nc.vector.memset(onesrow_st, 1.0)
        ones_row = consts.tile([1, 128], F32R)
        nc.vector.tensor_copy(ones_row[:], onesrow_st[:])

        def pe_transpose_1024(dst_sb, src_tiles):
            """PE-transpose eight [128,128] SBUF tiles through one 2-bank
            PSUM tile, then one DVE copy into dst_sb (with f32r rounding)."""
            tp = sp_ps.tile([128, 1024], F32, tag="sp")
            for j, src in enumerate(src_tiles):
                nc.tensor.matmul(tp[:, j * 128:(j + 1) * 128], src, ident_sb[:],
                                 is_transpose=True, start=(j % 4 == 0),
                                 stop=(j % 4 == 3))
            nc.vector.tensor_copy(dst_sb, tp[:])

        def make_final_A(E, v_r):
            """PV + denominator accumulation, PSUM->SBUF copies."""
            state = {}

            def fin_a():
                otsb = otsb_p.tile([128, S], F32)   # O^T unnormalized
                dn = dn_ps.tile([32, 512], F32)
                drow = drow_p.tile([1, 1024], F32)
                for qg in range(2):
                    q0g = qg * 512
                    ot = ot_ps.tile([128, 512], F32, tag="ot")
                    kts = [kt for kt in range(NT) if kt * 128 < q0g + 512]
                    mm = []
                    for i, kt in enumerate(kts):
                        k0 = kt * 128
                        q0 = max(k0, q0g)
                        # E strip slice for q in [q0, q0g+512)
                        ex = E[:, kt, q0 - k0:q0g + 512 - k0]
                        mm.append((i == 0, i == len(kts) - 1, kt, q0 - q0g, ex))
                    for st, sp_, kt, co, ex in mm:
                        nc.tensor.matmul(ot[:, co:512], v_r[:, kt, :], ex,
                                         start=st, stop=sp_)
                    for st, sp_, kt, co, ex in mm:
                        nc.tensor.matmul(dn[:, co:512], ones_sb[:], ex,
                                         start=st, stop=sp_)
                    nc.vector.tensor_copy(otsb[:, q0g:q0g + 512], ot[:])
                    # stash this q-group's denominator row; the next group's
                    # start=True matmul re-zeroes the bank
                    nc.vector.tensor_copy(drow[0:1, q0g:q0g + 512],
                                          dn[0:1, :])
                state["otsb"] = otsb
                state["drow"] = drow
            return fin_a, state

        def make_final_B(state, b, h):
            """O^T transpose back to [q,d], reciprocal, normalize, store."""
            def fin_b():
                otsb, drow = state["otsb"], state["drow"]
                tps = []
                for g in range(2):
                    tp = tp_ps.tile([128, 512], F32, tag="tp")
                    for j in range(4):
                        t = g * 4 + j
                        nc.tensor.matmul(tp[:, j * 128:(j + 1) * 128],
                                         otsb[:, t * 128:(t + 1) * 128],
                                         ident_sb[:], is_transpose=True,
                                         start=(j == 0), stop=(j == 3))
                    tps.append(tp)

                dt = ot_ps.tile([128, 512], F32, tag="ot")
                for j in range(NT):
                    nc.tensor.matmul(dt[:, j:j + 1],
                                     drow[0:1, j * 128:(j + 1) * 128],
                                     ident_sb[0:1, 0:1],
                                     is_transpose=True, start=(j == 0),
                                     stop=(j == NT - 1))
                rec = rec_p.tile([128, NT], F32)
                nc.vector.reciprocal(rec[:], dt[:, 0:NT])

                ofin = ofin_p.tile([128, NT, D], F32)
                for g in range(2):
                    base = rec[:, g * 4:g * 4 + 4]
                    rec_b = bass.AP(tensor=base.tensor, offset=base.offset,
                                    ap=list(base.ap) + [[0, 128]])
                    nc.vector.tensor_mul(
                        ofin[:, g * 4:(g + 1) * 4, :],
                        tps[g][:].rearrange("p (t f) -> p t f", t=4), rec_b)
                out_eng.dma_start(
                    out=out.ap()[b, :, h, :].rearrange("(t p) d -> p t d", p=128),
                    in_=ofin)
            return fin_b

        def load_batch(b, eng):
            k_nat = knat_p.tile([128, NT, D], F32)
            eng.dma_start(
                out=k_nat, in_=xk.ap()[b].rearrange("(t p) d -> p t d", p=128))
            # kv-cache K write straight from SBUF: depends on the load, so
            # the scheduler can't hoist it into the startup HBM window
            kv_eng.dma_start(
                out=kvk.ap()[b * S:(b + 1) * S, :].rearrange(
                    "(t p) d -> p t d", p=128),
                in_=k_nat[:])
            return k_nat

        def transpose_k(k_nat):
            ktr = kt_p.tile([128, S], F32R)  # K^T: [d, k]
            pe_transpose_1024(ktr[:], [k_nat[:, j, :] for j in range(NT)])
            return ktr

        def load_v(b):
            # V load + f32r cast deferred: not needed until this batch's
            # first finalize, and an early DMA would flood the startup HBM
            # window / block the in-order DVE queue
            v_stg = vstg_p.tile([128, NT, D], F32)
            nc.sync.dma_start(
                out=v_stg, in_=xv.ap()[b].rearrange("(t p) d -> p t d", p=128))
            v_r = v_p.tile([128, NT, D], F32R)
            nc.vector.tensor_copy(v_r[:], v_stg[:])
            kv_eng.dma_start(
                out=kvv.ap()[b * S:(b + 1) * S, :].rearrange(
                    "(t p) d -> p t d", p=128),
                in_=v_stg[:])
            return v_r

        def make_qtr(b, h):
            q_nat = qnat_p.tile([128, NT, D], F32)
            load_eng.dma_start(
                out=q_nat,
                in_=xq.ap()[b, :, h, :].rearrange("(t p) d -> p t d", p=128))
            qtr = qt_p.tile([128, S], F32R)  # Q^T: [d, q]
            pe_transpose_1024(qtr[:], [q_nat[:, j, :] for j in range(NT)])
            return qtr

        def emit_qk(ktr, qtr):
            # scores (transposed) + exp into packed E tile; strip kt covers
            # q in [k0, 1024), stored at E[:, kt, 0:W]
            E = e_p.tile([128, NT, 1024], F32R)
            for kt in range(NT):
                k0 = kt * 128
                W = 1024 - k0
                sp = sp_ps.tile([128, 1024], F32, tag="sp")
                for c0 in range(0, W, 512):  # pieces at psum bank boundary
                    w = min(512, W - c0)
                    nc.tensor.matmul(sp[:, c0:c0 + w], ktr[:, k0:k0 + 128],
                                     qtr[:, k0 + c0:k0 + c0 + w],
                                     start=True, stop=True)
                nc.scalar.activation(E[:, kt, 0:W], sp[:, 0:W], EXP,
                                     scale=SCALE)
            return E

        def emit_masks(E):
            # causal-mask the eight diagonal 128x128 blocks (two ops so
            # PV(qg0) doesn't wait on the kt>=4 exps)
            mask_eng = nc.gpsimd if MASK_GPSIMD else nc.vector
            mask_b4 = bass.AP(tensor=mask_sb.tensor, offset=mask_sb.offset,
                              ap=[mask_sb.ap[0], [0, 4], mask_sb.ap[1]])
            mask_eng.tensor_mul(E[:, 0:4, 0:128], E[:, 0:4, 0:128], mask_b4)
            mask_eng.tensor_mul(E[:, 4:8, 0:128], E[:, 4:8, 0:128], mask_b4)

        # Software pipeline, per head idx:
        #   PE order: QK(idx) | PV+dn(idx-1) | Qtrans(idx+1) | Otrans(idx-1)
        # so the DVE copies of finalize(idx-1) overlap PE transpose work and
        # the next head's Q^T cast overlaps the finalize matmuls.
        NH = B * GROUP
        ktrs = {0: transpose_k(load_batch(0, nc.sync))}
        k_next = None
        qtrs = {0: make_qtr(0, 0)}
        v_rs = {}
        a_pend, b_pend = [], []
        for idx in range(NH):
            b, h = divmod(idx, GROUP)
            if h == 0:
                v_rs[b] = load_v(b)
            if idx == 2 and B > 1:
                k_next = load_batch(1, nc.sync)

            if idx == 0:
                # head 1's Q^T up front so head 0's QK doesn't wait on the
                # Q^T cast with an empty pipeline
                qtrs[1] = make_qtr(0, 1)
            E = emit_qk(ktrs[b], qtrs.pop(idx))
            if a_pend:
                a_pend.pop(0)()
            if idx + 1 < NH and idx + 1 not in qtrs:
                nb = (idx + 1) // GROUP
                if nb != b:
                    ktrs[nb] = transpose_k(k_next)
                qtrs[idx + 1] = make_qtr(nb, (idx + 1) % GROUP)
            if b_pend:
                b_pend.pop(0)()
            emit_masks(E)

            fin_a, state = make_final_A(E, v_rs[b])
            a_pend.append(fin_a)
            b_pend.append(make_final_B(state, b, h))

        while a_pend:
            a_pend.pop(0)()
            b_pend.pop(0)()

    nc.compile()
    return nc


_NC = None


def _get_nc():
    global _NC
    if _NC is None:
        _NC = build_program()
    return _NC


def make_in_maps(xq, xk, xv):
    xq = np.ascontiguousarray(np.asarray(xq, dtype=np.float32))
    xk = np.ascontiguousarray(np.asarray(xk, dtype=np.float32))
    xv = np.ascontiguousarray(np.asarray(xv, dtype=np.float32))
    mask = np.triu(np.ones((128, 128), dtype=np.float32))  # mask[k,q]=1 iff q>=k
    ident = np.eye(128, dtype=np.float32)
    in_maps = []
    for m in range(N_CORES):
        in_maps.append({
            "xq_s": np.ascontiguousarray(xq[:, :, GROUP * m:GROUP * (m + 1), :]),
            "xk_s": np.ascontiguousarray(xk[:, :, m, :]),
            "xv_s": np.ascontiguousarray(xv[:, :, m, :]),
            "mask": mask,
            "ident": ident,
        })
    return in_maps


def assemble(results, kv_buffer, cur_select_index):
    out = np.empty((B, S, HQ, D), dtype=np.float32)
    kv_new = np.array(kv_buffer, dtype=np.float32, copy=True)
    idx = np.asarray(cur_select_index)
    for m in range(N_CORES):
        r = results[m]
        out[:, :, GROUP * m:GROUP * (m + 1), :] = r["out_s"]
        kv_new[idx, m, :] = r["kvk_s"]
        kv_new[idx, HKV + m, :] = r["kvv_s"]
    return out.reshape(B, S, HQ * D), kv_new


def kernel(xq, xk, xv, kv_buffer, cur_select_index):
    nc = _get_nc()
    in_maps = make_in_maps(xq, xk, xv)
    res = run_bass_kernel_spmd(nc, in_maps, core_ids=list(range(N_CORES)))
    return assemble(res.results, kv_buffer, cur_select_index)


# revision 39
# speedup vs baseline: 1.1010x; 1.0513x over previous
"""Trainium2 Bass kernel for GQA causal prefill attention + KV-cache write.

Problem shapes (hardcoded): xq [2,1024,32,128] f32, xk/xv [2,1024,8,128] f32,
kv_buffer [2048,16,128] f32, cur_select_index [2048] int (arange).
Reference computes:
  kv_buffer_new = kv_buffer.at[idx].set(concat(xk, xv, axis=2).reshape(-1,16,128))
  out = causal_softmax(QK^T/sqrt(d)) @ V with GQA (4 q-heads per kv head),
        returned as [2,1024,4096].

Sharding: 8 cores, core m handles q-heads 4m..4m+3 (both batches) and
kv-head m. Each core runs 8 independent (batch, head) attention problems.

Per-core dataflow (matmuls in float32r: ~tf32 accuracy at bf16 PE rate,
measured end-to-end relmax error ~2.5e-4 vs the fp32 reference):
  - PE-transpose Q,K tiles -> Q^T,K^T [d,s] in SBUF (contraction needs d on
    partitions).
  - Scores computed transposed: per k-tile kt, S^T[k,q] = K_kt @ Q^T for
    q in [128*kt, 1024) only (causal block skipping), into a [128,<=1024]
    PSUM strip split into <=512 matmul pieces at the bank boundary.
  - One exp per strip on ScalarE with 1/sqrt(d) folded into the activation
    scale, PSUM -> packed SBUF tile E[128, kt, 1024]; causal masking of all
    eight 128x128 diagonal blocks with a single VectorE multiply (diagonal
    blocks all sit at strip offset 0).
  - O^T[d,q] += V_kt^T @ expS^T_kt with V in natural layout as stationary;
    denominator via ones-vector matmuls, accumulated per 512-wide q-group.
  - Denominator row PE-transposed to column layout, reciprocal on VectorE,
    normalization fused into the PSUM->SBUF copy after PE-transposing O^T
    back to [q,d].
"""

from contextlib import ExitStack

import numpy as np

import concourse.bacc as bacc
import concourse.bass as bass
import concourse.mybir as mybir
import concourse.tile as tile
from concourse.bass_utils import run_bass_kernel_spmd

F32 = mybir.dt.float32
F32R = mybir.dt.float32r
BF16 = mybir.dt.bfloat16
EXP = mybir.ActivationFunctionType.Exp

B = 2
S = 1024
HQ = 32
HKV = 8
D = 128
GROUP = HQ // HKV  # q-heads per kv head = heads per core per batch
NT = S // 128      # 8 s-tiles
N_CORES = 8
SCALE = 1.0 / float(np.sqrt(D))

# experiment flags
BF16_IDENT = False    # bf16 identity in PE transposes: rejected for f32 data
BCAST_NORM = True     # stride-0 broadcast tensor_tensor for normalization
OUT_DMA = "sync"    # engine for output stores: "sync" | "scalar" | "gpsimd"
KV_DMA = "gpsimd"     # engine for kv dram->dram copies
LOAD_DMA = "sync"     # engine for input loads
STRIP1024 = True      # score strips [128,1024]x2 + 1 exp/kt; else [128,512]x3
MASK_GPSIMD = False   # run the causal mask multiply on gpsimd instead of DVE


def build_program():
    nc = bacc.Bacc("TRN2", target_bir_lowering=False, debug=False)

    xq = nc.dram_tensor("xq_s", [B, S, GROUP, D], F32, kind="ExternalInput")
    xk = nc.dram_tensor("xk_s", [B, S, D], F32, kind="ExternalInput")
    xv = nc.dram_tensor("xv_s", [B, S, D], F32, kind="ExternalInput")
    maskT = nc.dram_tensor("mask", [128, 128], F32, kind="ExternalInput")
    identT = nc.dram_tensor("ident", [128, 128], F32, kind="ExternalInput")
    out = nc.dram_tensor("out_s", [B, S, GROUP, D], F32, kind="ExternalOutput")
    kvk = nc.dram_tensor("kvk_s", [B * S, D], F32, kind="ExternalOutput")
    kvv = nc.dram_tensor("kvv_s", [B * S, D], F32, kind="ExternalOutput")

    IDT = BF16 if BF16_IDENT else F32

    with tile.TileContext(nc) as tc, ExitStack() as ctx:
        pool = lambda name, bufs, **kw: ctx.enter_context(
            tc.tile_pool(name=name, bufs=bufs, **kw))

        engs = {"sync": nc.sync, "scalar": nc.scalar, "gpsimd": nc.gpsimd}
        load_eng = engs[LOAD_DMA]
        kv_eng = engs[KV_DMA]
        out_eng = engs[OUT_DMA]

        consts = pool("consts", 1)
        knat_p = pool("knat", 1)
        vstg_p = pool("vstg", 1)
        v_p = pool("v", 2)
        kt_p = pool("kt", 2)
        qnat_p = pool("qnat", 2)
        qt_p = pool("qt", 2)
        e_p = pool("exps", 2)
        drow_p = pool("drow", 2)
        rec_p = pool("rec", 2)
        rb_p = pool("rb", 2)
        otsb_p = pool("otsb", 2)
        ofin_p = pool("ofin", 2)
        # PSUM budget is 8 banks:
        #   STRIP1024: sp 2x[128,1024]=4, tp 2x[128,512]=2, ot 1, dn 1
        #   else:      sp 3x[128,512]=3,  tp 2x[128,512]=2, ot 2, dn 1
        sp_ps = pool("sp", 2 if STRIP1024 else 3, space="PSUM")
        tp_ps = pool("tp", 2, space="PSUM")
        ot_ps = pool("ot", 1 if STRIP1024 else 2, space="PSUM")
        dn_ps = pool("dn", 1, space="PSUM")

        # --- PE warm-up: ~4us of dependency-free matmuls so the HAM clock
        # gate reaches K=8/8 before the real work arrives ---
        junk = consts.tile([128, 128], F32)
        nc.vector.memset(junk, 1.0)
        wu = sp_ps.tile([128, 1024], F32, tag="sp")
        for j in range(24):
            nc.tensor.matmul(wu[:, (j % 8) * 128:(j % 8) * 128 + 128],
                             junk[:], junk[:],
                             start=(j % 4 == 0), stop=(j % 4 == 3))

        # --- constants ---
        ident_sb = consts.tile([128, 128], F32)
        nc.sync.dma_start(out=ident_sb, in_=identT.ap())
        mask_sb = consts.tile([128, 128], F32)
        nc.sync.dma_start(out=mask_sb, in_=maskT.ap())
        ones_st = consts.tile([128, 32], F32)
        nc.vector.memset(ones_st, 1.0)
        ones_sb = consts.tile([128, 32], F32R)
        nc.vector.tensor_copy(ones_sb[:], ones_st[:])
        onesrow_st = consts.tile([1, 128], F32)
        nc.vector.memset(onesrow_st, 1.0)
        ones_row = consts.tile([1, 128], F32R)
        nc.vector.tensor_copy(ones_row[:], onesrow_st[:])

        def pe_transpose_1024(dst_sb, src_tiles):
            """PE-transpose eight [128,128] SBUF tiles through one 2-bank
            PSUM tile, then one DVE copy into dst_sb (with f32r rounding)."""
            tp = sp_ps.tile([128, 1024], F32, tag="sp")
            for j, src in enumerate(src_tiles):
                nc.tensor.matmul(tp[:, j * 128:(j + 1) * 128], src, ident_sb[:],
                                 is_transpose=True, start=(j % 4 == 0),
                                 stop=(j % 4 == 3))
            nc.vector.tensor_copy(dst_sb, tp[:])

        def make_final_A(E, v_r):
            """PV + denominator accumulation, PSUM->SBUF copies."""
            state = {}

            def fin_a():
                otsb = otsb_p.tile([128, S], F32)   # O^T unnormalized
                dn = dn_ps.tile([32, 512], F32)
                drow = drow_p.tile([1, 1024], F32)
                for qg in range(2):
                    q0g = qg * 512
                    ot = ot_ps.tile([128, 512], F32, tag="ot")
                    kts = [kt for kt in range(NT) if kt * 128 < q0g + 512]
                    mm = []
                    for i, kt in enumerate(kts):
                        k0 = kt * 128
                        q0 = max(k0, q0g)
                        # E strip slice for q in [q0, q0g+512)
                        ex = E[:, kt, q0 - k0:q0g + 512 - k0]
                        mm.append((i == 0, i == len(kts) - 1, kt, q0 - q0g, ex))
                    for st, sp_, kt, co, ex in mm:
                        nc.tensor.matmul(ot[:, co:512], v_r[:, kt, :], ex,
                                         start=st, stop=sp_)
                    for st, sp_, kt, co, ex in mm:
                        nc.tensor.matmul(dn[:, co:512], ones_sb[:], ex,
                                         start=st, stop=sp_)
                    nc.vector.tensor_copy(otsb[:, q0g:q0g + 512], ot[:])
                    # stash this q-group's denominator row; the next group's
                    # start=True matmul re-zeroes the bank
                    nc.vector.tensor_copy(drow[0:1, q0g:q0g + 512],
                                          dn[0:1, :])
                state["otsb"] = otsb
                state["drow"] = drow
            return fin_a, state

        def make_final_B(state, b, h):
            """O^T transpose back to [q,d], reciprocal, normalize, store."""
            def fin_b():
                otsb, drow = state["otsb"], state["drow"]
                tps = []
                for g in range(2):
                    tp = tp_ps.tile([128, 512], F32, tag="tp")
                    for j in range(4):
                        t = g * 4 + j
                        nc.tensor.matmul(tp[:, j * 128:(j + 1) * 128],
                                         otsb[:, t * 128:(t + 1) * 128],
                                         ident_sb[:], is_transpose=True,
                                         start=(j == 0), stop=(j == 3))
                    tps.append(tp)

                dt = ot_ps.tile([128, 512], F32, tag="ot")
                for j in range(NT):
                    nc.tensor.matmul(dt[:, j:j + 1],
                                     drow[0:1, j * 128:(j + 1) * 128],
                                     ident_sb[0:1, 0:1],
                                     is_transpose=True, start=(j == 0),
                                     stop=(j == NT - 1))
                rec = rec_p.tile([128, NT], F32)
                nc.vector.reciprocal(rec[:], dt[:, 0:NT])

                ofin = ofin_p.tile([128, NT, D], F32)
                for g in range(2):
                    base = rec[:, g * 4:g * 4 + 4]
                    rec_b = bass.AP(tensor=base.tensor, offset=base.offset,
                                    ap=list(base.ap) + [[0, 128]])
                    nc.vector.tensor_mul(
                        ofin[:, g * 4:(g + 1) * 4, :],
                        tps[g][:].rearrange("p (t f) -> p t f", t=4), rec_b)
                out_eng.dma_start(
                    out=out.ap()[b, :, h, :].rearrange("(t p) d -> p t d", p=128),
                    in_=ofin)
            return fin_b

        def load_batch(b, eng):
            k_nat = knat_p.tile([128, NT, D], F32)
            eng.dma_start(
                out=k_nat, in_=xk.ap()[b].rearrange("(t p) d -> p t d", p=128))
            # kv-cache K write straight from SBUF: depends on the load, so
            # the scheduler can't hoist it into the startup HBM window
            kv_eng.dma_start(
                out=kvk.ap()[b * S:(b + 1) * S, :].rearrange(
                    "(t p) d -> p t d", p=128),
                in_=k_nat[:])
            return k_nat

        def transpose_k(k_nat):
            ktr = kt_p.tile([128, S], F32R)  # K^T: [d, k]
            pe_transpose_1024(ktr[:], [k_nat[:, j, :] for j in range(NT)])
            return ktr

        def load_v(b):
            # V load + f32r cast deferred: not needed until this batch's
            # first finalize, and an early DMA would flood the startup HBM
            # window / block the in-order DVE queue
            v_stg = vstg_p.tile([128, NT, D], F32)
            nc.scalar.dma_start(
                out=v_stg, in_=xv.ap()[b].rearrange("(t p) d -> p t d", p=128))
            v_r = v_p.tile([128, NT, D], F32R)
            nc.vector.tensor_copy(v_r[:], v_stg[:])
            kv_eng.dma_start(
                out=kvv.ap()[b * S:(b + 1) * S, :].rearrange(
                    "(t p) d -> p t d", p=128),
                in_=v_stg[:])
            return v_r

        def make_qtr(b, h):
            q_nat = qnat_p.tile([128, NT, D], F32)
            load_eng.dma_start(
                out=q_nat,
                in_=xq.ap()[b, :, h, :].rearrange("(t p) d -> p t d", p=128))
            qtr = qt_p.tile([128, S], F32R)  # Q^T: [d, q]
            pe_transpose_1024(qtr[:], [q_nat[:, j, :] for j in range(NT)])
            return qtr

        def emit_qk(ktr, qtr):
            # scores (transposed) + exp into packed E tile; strip kt covers
            # q in [k0, 1024), stored at E[:, kt, 0:W]
            E = e_p.tile([128, NT, 1024], F32R)
            for kt in range(NT):
                k0 = kt * 128
                W = 1024 - k0
                sp = sp_ps.tile([128, 1024], F32, tag="sp")
                for c0 in range(0, W, 512):  # pieces at psum bank boundary
                    w = min(512, W - c0)
                    nc.tensor.matmul(sp[:, c0:c0 + w], ktr[:, k0:k0 + 128],
                                     qtr[:, k0 + c0:k0 + c0 + w],
                                     start=True, stop=True)
                nc.scalar.activation(E[:, kt, 0:W], sp[:, 0:W], EXP,
                                     scale=SCALE)
            return E

        def emit_masks(E):
            # causal-mask the eight diagonal 128x128 blocks (two ops so
            # PV(qg0) doesn't wait on the kt>=4 exps)
            mask_eng = nc.gpsimd if MASK_GPSIMD else nc.vector
            mask_b4 = bass.AP(tensor=mask_sb.tensor, offset=mask_sb.offset,
                              ap=[mask_sb.ap[0], [0, 4], mask_sb.ap[1]])
            mask_eng.tensor_mul(E[:, 0:4, 0:128], E[:, 0:4, 0:128], mask_b4)
            mask_eng.tensor_mul(E[:, 4:8, 0:128], E[:, 4:8, 0:128], mask_b4)

        # Software pipeline, per head idx:
        #   PE order: QK(idx) | PV+dn(idx-1) | Qtrans(idx+1) | Otrans(idx-1)
        # so the DVE copies of finalize(idx-1) overlap PE transpose work and
        # the next head's Q^T cast overlaps the finalize matmuls.
        NH = B * GROUP
        ktrs = {0: transpose_k(load_batch(0, nc.sync))}
        k_next = None
        qtrs = {0: make_qtr(0, 0)}
        v_rs = {}
        a_pend, b_pend = [], []
        for idx in range(NH):
            b, h = divmod(idx, GROUP)
            if h == 0:
                v_rs[b] = load_v(b)
            if idx == 2 and B > 1:
                k_next = load_batch(1, nc.sync)

            if idx == 0:
                # head 1's Q^T up front so head 0's QK doesn't wait on the
                # Q^T cast with an empty pipeline
                qtrs[1] = make_qtr(0, 1)
            E = emit_qk(ktrs[b], qtrs.pop(idx))
            if a_pend:
                a_pend.pop(0)()
            if idx + 1 < NH and idx + 1 not in qtrs:
                nb = (idx + 1) // GROUP
                if nb != b:
                    ktrs[nb] = transpose_k(k_next)
                qtrs[idx + 1] = make_qtr(nb, (idx + 1) % GROUP)
            if b_pend:
                b_pend.pop(0)()
            emit_masks(E)

            fin_a, state = make_final_A(E, v_rs[b])
            a_pend.append(fin_a)
            b_pend.append(make_final_B(state, b, h))

        while a_pend:
            a_pend.pop(0)()
            b_pend.pop(0)()

    nc.compile()
    return nc


_NC = None


def _get_nc():
    global _NC
    if _NC is None:
        _NC = build_program()
    return _NC


def make_in_maps(xq, xk, xv):
    xq = np.ascontiguousarray(np.asarray(xq, dtype=np.float32))
    xk = np.ascontiguousarray(np.asarray(xk, dtype=np.float32))
    xv = np.ascontiguousarray(np.asarray(xv, dtype=np.float32))
    mask = np.triu(np.ones((128, 128), dtype=np.float32))  # mask[k,q]=1 iff q>=k
    ident = np.eye(128, dtype=np.float32)
    in_maps = []
    for m in range(N_CORES):
        in_maps.append({
            "xq_s": np.ascontiguousarray(xq[:, :, GROUP * m:GROUP * (m + 1), :]),
            "xk_s": np.ascontiguousarray(xk[:, :, m, :]),
            "xv_s": np.ascontiguousarray(xv[:, :, m, :]),
            "mask": mask,
            "ident": ident,
        })
    return in_maps


def assemble(results, kv_buffer, cur_select_index):
    out = np.empty((B, S, HQ, D), dtype=np.float32)
    kv_new = np.array(kv_buffer, dtype=np.float32, copy=True)
    idx = np.asarray(cur_select_index)
    for m in range(N_CORES):
        r = results[m]
        out[:, :, GROUP * m:GROUP * (m + 1), :] = r["out_s"]
        kv_new[idx, m, :] = r["kvk_s"]
        kv_new[idx, HKV + m, :] = r["kvv_s"]
    return out.reshape(B, S, HQ * D), kv_new


def kernel(xq, xk, xv, kv_buffer, cur_select_index):
    nc = _get_nc()
    in_maps = make_in_maps(xq, xk, xv)
    res = run_bass_kernel_spmd(nc, in_maps, core_ids=list(range(N_CORES)))
    return assemble(res.results, kv_buffer, cur_select_index)
